# revision 11
# baseline (speedup 1.0000x reference)
"""Trainium2 Bass kernel for the Spikformer-style spiking attention block.

Problem: nn_Attention_76776835383802
  x: (T=4, B=8, C=512, N=1024) f32
  out = lif(bn(wp @ a_sp + pb)), v = heads(lif(bn(wv @ x)))

Sharding: data-parallel over B — core b processes batch b (the LIF recurrence
is only over T; B shards cleanly).

Per-core pipeline (per timestep t):
  - x split into fp16 hi/lo per 128-channel chunk (ACT cast + GPSIMD subtract)
  - q-conv natural layout (C_out on partitions): fp16 hi/lo 3-pass matmul,
    which carries fp32-grade accuracy at 1 cycle/row
  - k/v-convs transposed layout (N on partitions) so attention needs no
    transposes; BN bias added via broadcast-tile DVE ops
  - LIF via m=2v recurrence (bitwise-equal to reference):
    s=[m>=2*theta]; u=[m<2*theta]*m (STT, in-place); m' = 0.5*u (+bias) + psum
  - attention: spikes exact in fp16; kv = k^T v packed 2 heads per 128x128
    matmul (cross blocks ignored); a^T = (kv*scale)^T q via tile_position
    quadrant packing; integer-exact throughout
  - p-conv natural, 2-pass (spikes exact in fp16), BN+pb bias per-partition
"""
import sys
sys.path.insert(0, '/opt/trn_rl_repo')
import os
import numpy as np

T, B, C, N = 4, 8, 512, 1024
H, D = 8, 64
P = 128
KC = C // P      # 4 contraction chunks
MC = C // P      # 4 output-channel chunks
NCH = N // P     # 8 token chunks
NH = N // 512    # 2 free-dim halves

_CACHE = {}


def _build():
    import concourse.bacc as bacc
    import concourse.mybir as mybir
    from concourse.tile import TileContext
    from concourse.alu_op_type import AluOpType as Op

    f32 = mybir.dt.float32
    f16 = mybir.dt.float16
    ACT = mybir.ActivationFunctionType
    REPS = int(os.environ.get("KREPS", "1"))

    nc = bacc.Bacc("TRN2", target_bir_lowering=False)

    x_d = nc.dram_tensor("x", (T, C, N), f32, kind="ExternalInput")
    w_d = {}
    for nm in ("wqh", "wql", "wkh", "wkl", "wvh", "wvl", "wph", "wpl"):
        w_d[nm] = nc.dram_tensor(nm, (C, C), f16, kind="ExternalInput")
    bq_d = nc.dram_tensor("bq", (C,), f32, kind="ExternalInput")
    bp_d = nc.dram_tensor("bp", (C,), f32, kind="ExternalInput")
    bk_d = nc.dram_tensor("bkt", (P, C), f32, kind="ExternalInput")
    bv_d = nc.dram_tensor("bvt", (P, C), f32, kind="ExternalInput")
    out_d = nc.dram_tensor("out", (T, C, N), f32, kind="ExternalOutput")
    v_d = nc.dram_tensor("v", (T, H, N, D), f32, kind="ExternalOutput")

    with TileContext(nc) as tc:
        with tc.tile_pool(name="wp_", bufs=1) as wpool, \
             tc.tile_pool(name="cst", bufs=1) as cst, \
             tc.tile_pool(name="st", bufs=1) as st, \
             tc.tile_pool(name="xin", bufs=5) as xin, \
             tc.tile_pool(name="xhp", bufs=5) as xhp, \
             tc.tile_pool(name="xlp", bufs=5) as xlp, \
             tc.tile_pool(name="spk", bufs=1) as spk, \
             tc.tile_pool(name="kvp_", bufs=2) as kvpool, \
             tc.tile_pool(name="sout_", bufs=2) as soutp, \
             tc.tile_pool(name="psc", bufs=5, space="PSUM") as psc, \
             tc.tile_pool(name="pskv", bufs=1, space="PSUM") as pskv, \
             tc.tile_pool(name="psa", bufs=2, space="PSUM") as psa:

            # ---- x loader: per-chunk DMA + fp16 hi/lo split ----
            def load_split_x(t, tag):
                xhs, xls = [], []
                for kc in range(KC):
                    xf = xin.tile([P, N], f32, tag="xf", name=f"xf{tag}_{kc}")
                    nc.sync.dma_start(
                        xf[:], x_d[t].rearrange("(k p) n -> k p n", p=P)[kc])
                    xh = xhp.tile([P, N], f16, tag="xh", name=f"xh{tag}_{kc}")
                    xl = xlp.tile([P, N], f16, tag="xl", name=f"xl{tag}_{kc}")
                    nc.scalar.activation(xh[:], xf[:], ACT.Copy)
                    nc.gpsimd.tensor_tensor(xl[:], xf[:], xh[:], Op.subtract)
                    xhs.append(xh)
                    xls.append(xl)
                return xhs, xls

            # ---- weights (q first), x[0] interleaved so PE can start ASAP ----
            w = {}
            def load_w(nm):
                w[nm] = wpool.tile([P, KC, C], f16, tag=nm, name=f"w_{nm}")
                nc.sync.dma_start(w[nm][:], w_d[nm].rearrange("(k p) o -> p k o", p=P))
            load_w("wqh")
            load_w("wql")
            xsplit0 = load_split_x(0, "00")
            for nm in ("wkh", "wkl", "wvh", "wvl", "wph", "wpl"):
                load_w(nm)
            bq = cst.tile([P, MC], f32, tag="bq", name="bq_t")
            bp = cst.tile([P, MC], f32, tag="bp", name="bp_t")
            nc.sync.dma_start(bq[:], bq_d.rearrange("(k p) -> p k", p=P))
            nc.sync.dma_start(bp[:], bp_d.rearrange("(k p) -> p k", p=P))
            bkt = cst.tile([P, C], f32, tag="bkt", name="bkt_t")
            bvt = cst.tile([P, C], f32, tag="bvt", name="bvt_t")
            nc.sync.dma_start(bkt[:], bk_d[:])
            nc.sync.dma_start(bvt[:], bv_d[:])

            # ---- persistent LIF states (m = 2*v, in-place updated) ----
            m_q = st.tile([P, MC, N], f32, tag="m_q", name="m_q")
            m_k = st.tile([P, NCH, C], f32, tag="m_k", name="m_k")
            m_v = st.tile([P, NCH, C], f32, tag="m_v", name="m_v")
            m_a = st.tile([P, MC, N], f32, tag="m_a", name="m_a")
            m_p = st.tile([P, MC, N], f32, tag="m_p", name="m_p")

            for rep in range(REPS):
              for t in range(T):
                if rep == 0 and t == 0:
                    xhs, xls = xsplit0
                else:
                    xhs, xls = load_split_x(t, f"{rep}{t}")

                # ---- q conv (natural, 3-pass) + LIF ----
                q_sp = spk.tile([P, MC, N], f16, tag="q_sp", name=f"q_sp{rep}{t}")
                for m in range(MC):
                    for nh in range(NH):
                        ps = psc.tile([P, 512], f32, tag="psc", name=f"qps{rep}{t}_{m}_{nh}")
                        i = 0
                        for wt, xt in ((w["wqh"], xhs), (w["wqh"], xls),
                                       (w["wql"], xhs)):
                            for kc in range(KC):
                                nc.tensor.matmul(
                                    ps[:], wt[:, kc, m * P:(m + 1) * P],
                                    xt[kc][:, nh * 512:(nh + 1) * 512],
                                    start=(i == 0), stop=(i == 11))
                                i += 1
                        msl = m_q[:, m, nh * 512:(nh + 1) * 512]
                        if t == 0:
                            nc.vector.tensor_scalar(msl, ps[:], bq[:, m:m + 1], None, Op.add)
                        else:
                            nc.vector.tensor_tensor(msl, msl, ps[:], Op.add)
                        nc.vector.tensor_scalar(
                            q_sp[:, m, nh * 512:(nh + 1) * 512], msl, 2.0, None, Op.is_ge)
                        if t < T - 1:
                            nc.vector.scalar_tensor_tensor(msl, msl, 2.0, msl, Op.is_lt, Op.mult)
                            nc.vector.tensor_scalar(msl, msl, 0.5, bq[:, m:m + 1], Op.mult, Op.add)

                # ---- k/v convs (transposed, 3-pass) + LIF ----
                k_sp = spk.tile([P, NCH, C], f16, tag="k_sp", name=f"k_sp{rep}{t}")
                v_sp = spk.tile([P, NCH, C], f16, tag="v_sp", name=f"v_sp{rep}{t}")
                for nm, wh, wl, btile, sp, mt in (
                        ("k", w["wkh"], w["wkl"], bkt, k_sp, m_k),
                        ("v", w["wvh"], w["wvl"], bvt, v_sp, m_v)):
                    for nch in range(NCH):
                        ps = psc.tile([P, 512], f32, tag="psc", name=f"{nm}ps{rep}{t}_{nch}")
                        i = 0
                        for kc in range(KC):
                            xsl = xhs[kc][:, nch * P:(nch + 1) * P]
                            xll = xls[kc][:, nch * P:(nch + 1) * P]
                            for lhs, rhs in ((xsl, wh), (xsl, wl), (xll, wh)):
                                nc.tensor.matmul(
                                    ps[:], lhs, rhs[:, kc, :],
                                    start=(i == 0), stop=(i == 11))
                                i += 1
                        msl = mt[:, nch, :]
                        if t == 0:
                            nc.vector.tensor_tensor(msl, ps[:], btile[:], Op.add)
                        else:
                            nc.vector.tensor_tensor(msl, msl, ps[:], Op.add)
                        nc.vector.tensor_scalar(sp[:, nch, :], msl, 2.0, None, Op.is_ge)
                        if t < T - 1:
                            nc.vector.scalar_tensor_tensor(msl, msl, 2.0, msl, Op.is_lt, Op.mult)
                            nc.vector.scalar_tensor_tensor(msl, msl, 0.5, btile[:], Op.mult, Op.add)

                # v export with f16->f32 cast dma (gpsimd)
                for nch in range(NCH):
                    nc.gpsimd.dma_start(
                        v_d[t].rearrange("h (c p) d -> c p h d", p=P)[nch],
                        v_sp[:, nch, :].rearrange("p (h d) -> p h d", d=D))

                # ---- attention + attn-LIF ----
                a_sp = spk.tile([P, MC, N], f16, tag="a_sp", name=f"a_sp{rep}{t}")
                for pr in range(4):  # head pairs
                    kv_p = pskv.tile([P, P], f32, tag="pskv", name=f"kvps{rep}{t}_{pr}")
                    for nch in range(NCH):
                        nc.tensor.matmul(
                            kv_p[:], k_sp[:, nch, pr * P:(pr + 1) * P],
                            v_sp[:, nch, pr * P:(pr + 1) * P],
                            start=(nch == 0), stop=(nch == NCH - 1))
                    kvt = kvpool.tile([P, D], f16, tag="kvt", name=f"kvt{rep}{t}_{pr}")
                    # diagonal head blocks only, scaled by 0.125 (exact in f16)
                    nc.scalar.activation(kvt[0:64, :], kv_p[0:64, 0:64], ACT.Copy, scale=0.125)
                    nc.scalar.activation(kvt[64:128, :], kv_p[64:128, 64:128], ACT.Copy, scale=0.125)
                    for nh in range(NH):
                        ps = psa.tile([P, 512], f32, tag="psa", name=f"aps{rep}{t}_{pr}_{nh}")
                        nc.tensor.matmul(
                            ps[0:64, :], kvt[0:64, :],
                            q_sp[0:64, pr, nh * 512:(nh + 1) * 512],
                            start=True, stop=True, tile_position=(0, 0))
                        nc.tensor.matmul(
                            ps[64:128, :], kvt[64:128, :],
                            q_sp[64:128, pr, nh * 512:(nh + 1) * 512],
                            start=True, stop=True, tile_position=(64, 64))
                        msl = m_a[:, pr, nh * 512:(nh + 1) * 512]
                        if t == 0:
                            nc.scalar.activation(msl, ps[:], ACT.Copy)
                        else:
                            nc.vector.scalar_tensor_tensor(msl, msl, 0.5, ps[:], Op.mult, Op.add)
                        nc.vector.tensor_scalar(
                            a_sp[:, pr, nh * 512:(nh + 1) * 512], msl, 1.0, None, Op.is_ge)
                        if t < T - 1:
                            nc.vector.scalar_tensor_tensor(msl, msl, 1.0, msl, Op.is_lt, Op.mult)

                # ---- p conv (natural, 2-pass) + out LIF ----
                s_out = soutp.tile([P, MC, N], f16, tag="s_out", name=f"s_out{rep}{t}")
                for m in range(MC):
                    for nh in range(NH):
                        ps = psc.tile([P, 512], f32, tag="psc", name=f"pps{rep}{t}_{m}_{nh}")
                        i = 0
                        for wt in (w["wph"], w["wpl"]):
                            for kc in range(KC):
                                nc.tensor.matmul(
                                    ps[:], wt[:, kc, m * P:(m + 1) * P],
                                    a_sp[:, kc, nh * 512:(nh + 1) * 512],
                                    start=(i == 0), stop=(i == 7))
                                i += 1
                        msl = m_p[:, m, nh * 512:(nh + 1) * 512]
                        if t == 0:
                            nc.vector.tensor_scalar(msl, ps[:], bp[:, m:m + 1], None, Op.add)
                        else:
                            nc.vector.tensor_tensor(msl, msl, ps[:], Op.add)
                        nc.vector.tensor_scalar(
                            s_out[:, m, nh * 512:(nh + 1) * 512], msl, 2.0, None, Op.is_ge)
                        if t < T - 1:
                            nc.vector.scalar_tensor_tensor(msl, msl, 2.0, msl, Op.is_lt, Op.mult)
                            nc.vector.tensor_scalar(msl, msl, 0.5, bp[:, m:m + 1], Op.mult, Op.add)

                # out export with f16->f32 cast dma (gpsimd), per chunk
                for m in range(MC):
                    nc.gpsimd.dma_start(
                        out_d[t].rearrange("(k p) n -> k p n", p=P)[m],
                        s_out[:, m, :])

    nc.compile()
    return nc


def _prepare_host(inputs):
    """Fold BN into weights, build fp16 hi/lo splits and bias tiles."""
    eps = 1e-5

    def inv_shift(g, b, m, var):
        inv = g.astype(np.float64) / np.sqrt(var.astype(np.float64) + eps)
        shift = b.astype(np.float64) - m.astype(np.float64) * inv
        return inv, shift

    def split16(a64):
        hi = a64.astype(np.float16)
        lo = (a64 - hi.astype(np.float64)).astype(np.float16)
        return np.ascontiguousarray(hi), np.ascontiguousarray(lo)

    args = {}
    iq, sq = inv_shift(inputs["gq"], inputs["bq"], inputs["mq"], inputs["vq"])
    ik, sk = inv_shift(inputs["gk"], inputs["bk"], inputs["mk"], inputs["vk"])
    iv, sv = inv_shift(inputs["gv"], inputs["bv"], inputs["mv"], inputs["vv"])
    ip, sp_ = inv_shift(inputs["gp"], inputs["bp"], inputs["mp"], inputs["vp"])

    for nm, wmat, inv in (("wq", inputs["wq"], iq), ("wk", inputs["wk"], ik),
                          ("wv", inputs["wv"], iv)):
        wT = (wmat.astype(np.float64) * inv[:, None]).T  # (c_in, c_out)
        hi, lo = split16(wT)
        args[nm + "h"], args[nm + "l"] = hi, lo
    wpT = (inputs["wp"].astype(np.float64) * ip[:, None]).T
    args["wph"], args["wpl"] = split16(wpT)

    args["bq"] = sq.astype(np.float32)
    # p-branch bias folds conv bias pb through BN: inv*(p + pb) + shift
    args["bp"] = (ip * inputs["pb"].astype(np.float64) + sp_).astype(np.float32)
    # k/v biases replicated across partitions (free-dim = channel)
    args["bkt"] = np.ascontiguousarray(
        np.broadcast_to(sk.astype(np.float32), (P, C)))
    args["bvt"] = np.ascontiguousarray(
        np.broadcast_to(sv.astype(np.float32), (P, C)))
    return args


def kernel(**inputs):
    from concourse.bass_utils import run_bass_kernel_spmd

    if "nc" not in _CACHE:
        _CACHE["nc"] = _build()
    nc = _CACHE["nc"]

    shared = _prepare_host(inputs)
    x = inputs["x"]
    in_maps = []
    for b in range(B):
        m = dict(shared)
        m["x"] = np.ascontiguousarray(x[:, b])
        in_maps.append(m)

    res = run_bass_kernel_spmd(nc, in_maps, core_ids=list(range(B)))
    out = np.empty((T, B, C, N), np.float32)
    v = np.empty((T, B, H, N, D), np.float32)
    for b in range(B):
        out[:, b] = res.results[b]["out"]
        v[:, b] = res.results[b]["v"]
    return out, v
